# revision 8
# baseline (speedup 1.0000x reference)
"""Baseline kernel (restored from session start) — control experiment."""

import numpy as np

B, T, H = 32, 4096, 512
NCORES = 8
TC = T // NCORES          # 512 t-values per core
P = 128                   # partitions
NT = (TC * B) // 512      # 32 n-tiles of 512 (t,b) pairs
TPT = 512 // B            # 16 t-values per n-tile
KC = H // P               # 4 k-chunks
MC = H // P               # 4 m-chunks (h_out)
NTG = 4                   # n-tiles per scores psum bank
SCW = NTG * P             # scores columns per bank (512)

_CACHE = {}


def _build_nc():
    import concourse.mybir as mybir
    from concourse.bacc import Bacc
    from concourse.tile import TileContext

    f32 = mybir.dt.float32
    bf16 = mybir.dt.bfloat16
    AF = mybir.ActivationFunctionType
    AX = mybir.AxisListType

    nc = Bacc()

    encT = nc.declare_dram_parameter("enct", [H, TC * B], bf16, isOutput=False)
    w2t = nc.declare_dram_parameter("w2t", [H, H], bf16, isOutput=False)
    w1t = nc.declare_dram_parameter("w1t", [H, H], bf16, isOutput=False)
    hidT = nc.declare_dram_parameter("hidt", [H, B], bf16, isOutput=False)
    brow = nc.declare_dram_parameter("brow", [1, H], bf16, isOutput=False)
    ones = nc.declare_dram_parameter("ones", [1, B], bf16, isOutput=False)
    ind = nc.declare_dram_parameter("ind", [P, 512], bf16, isOutput=False)
    vcol = nc.declare_dram_parameter("vcol", [P, MC], f32, isOutput=False)
    out = nc.declare_dram_parameter("scores", [16, NT // NTG * SCW], f32,
                                    isOutput=True)

    with TileContext(nc) as tc:
        with (
            tc.tile_pool(name="consts", bufs=1) as consts,
            tc.tile_pool(name="enc", bufs=NT * KC) as encp,
            tc.tile_pool(name="xs", bufs=12) as xp,
            tc.tile_pool(name="dens", bufs=24) as dp,
            tc.tile_pool(name="mainps", bufs=4, space="PSUM") as psp,
            tc.tile_pool(name="scps", bufs=3, space="PSUM") as scp,
            tc.tile_pool(name="apps", bufs=1, space="PSUM") as app,
        ):
            w2t_sb, w1t_sb, hid_sb = [], [], []
            for kc in range(KC):
                t_ = consts.tile([P, H], bf16, name=f"w2t{kc}")
                nc.sync.dma_start(out=t_, in_=w2t[kc * P:(kc + 1) * P, :])
                w2t_sb.append(t_)
                t_ = consts.tile([P, H], bf16, name=f"w1t{kc}")
                nc.sync.dma_start(out=t_, in_=w1t[kc * P:(kc + 1) * P, :])
                w1t_sb.append(t_)
                t_ = consts.tile([P, B], bf16, name=f"hidt{kc}")
                nc.sync.dma_start(out=t_, in_=hidT[kc * P:(kc + 1) * P, :])
                hid_sb.append(t_)
            brow_sb = consts.tile([1, H], bf16, name="brow")
            nc.sync.dma_start(out=brow_sb, in_=brow[:, :])
            ones_sb = consts.tile([1, B], bf16, name="ones")
            nc.sync.dma_start(out=ones_sb, in_=ones[:, :])
            ind_sb = consts.tile([P, 512], bf16, name="ind")
            nc.sync.dma_start(out=ind_sb, in_=ind[:, :])
            vcol_sb = consts.tile([P, MC], f32, name="vcol")
            nc.sync.dma_start(out=vcol_sb, in_=vcol[:, :])
            scores_sb = consts.tile([P, NT // NTG * SCW], f32,
                                    name="scores_sb")
            warm = consts.tile([1, 1], f32, name="actwarm")
            nc.scalar.activation(out=warm, in_=vcol_sb[0:1, 0:1], func=AF.Exp)

            ap_ps = app.tile([P, H], f32, name="ap_ps")
            for g in range(4):
                for kc in range(KC):
                    nc.tensor.matmul(
                        out=ap_ps[32 * g:32 * (g + 1), :],
                        lhsT=hid_sb[kc], rhs=w1t_sb[kc],
                        start=(kc == 0), stop=False,
                        tile_position=(0, 32 * g),
                    )
                nc.tensor.matmul(
                    out=ap_ps[32 * g:32 * (g + 1), :],
                    lhsT=ones_sb, rhs=brow_sb, start=False, stop=True,
                    tile_position=(0, 32 * g),
                )
            aprep_sb = consts.tile([P, H], bf16, name="aprep")
            nc.vector.tensor_copy(out=aprep_sb, in_=ap_ps)

            for nt in range(NT):
                e_sb = []
                for kc in range(KC):
                    t_ = encp.tile([P, 512], bf16, tag="enc")
                    nc.sync.dma_start(
                        out=t_,
                        in_=encT[kc * P:(kc + 1) * P, nt * 512:(nt + 1) * 512],
                    )
                    e_sb.append(t_)

                sc_ps = scp.tile([P, P], f32, tag="scps")

                ps_tiles = []
                for mc in range(MC):
                    ps = psp.tile([P, 512], f32, tag="mainps")
                    for kc in range(KC):
                        nc.tensor.matmul(
                            out=ps,
                            lhsT=w2t_sb[kc][:, mc * P:(mc + 1) * P],
                            rhs=e_sb[kc],
                            start=(kc == 0), stop=False,
                        )
                    ps_tiles.append(ps)
                for mc in range(MC):
                    nc.tensor.matmul(
                        out=ps_tiles[mc],
                        lhsT=aprep_sb[32 * mc:32 * (mc + 1),
                                      mc * P:(mc + 1) * P],
                        rhs=ind_sb[32 * mc:32 * (mc + 1), :],
                        start=False, stop=True,
                        tile_position=(32 * mc, 0),
                    )

                x_tiles, u_tiles = [], []
                for mc in range(MC):
                    x = xp.tile([P, 512], bf16, tag="x")
                    nc.scalar.activation(out=x, in_=ps_tiles[mc], func=AF.Exp)
                    x3 = x.rearrange("p (t b) -> p t b", b=B)

                    den = dp.tile([P, TPT], f32, tag="den")
                    nc.vector.reduce_sum(out=den, in_=x3, axis=AX.X)
                    rden = dp.tile([P, TPT], f32, tag="rden")
                    nc.vector.reciprocal(out=rden, in_=den)
                    u = dp.tile([P, TPT], bf16, tag="u")
                    nc.vector.tensor_scalar_mul(
                        out=u, in0=rden, scalar1=vcol_sb[:, mc:mc + 1],
                    )
                    x_tiles.append(x)
                    u_tiles.append(u)

                for r in range(4):
                    for mc in range(MC):
                        nc.tensor.matmul(
                            out=sc_ps[32 * r:32 * r + 4, :],
                            lhsT=u_tiles[mc][:, 4 * r:4 * (r + 1)],
                            rhs=x_tiles[mc][:, P * r:P * (r + 1)],
                            start=(mc == 0), stop=(mc == 3),
                            tile_position=(0, 32 * r),
                        )

                for r in range(4):
                    nc.scalar.activation(
                        out=scores_sb[32 * r:32 * r + 4, nt * P:(nt + 1) * P],
                        in_=sc_ps[32 * r:32 * r + 4, :], func=AF.Relu,
                    )

            for r in range(4):
                nc.gpsimd.dma_start(
                    out=out[4 * r:4 * (r + 1), :],
                    in_=scores_sb[32 * r:32 * r + 4, :],
                )

    nc.compile()
    return nc


def _prep_inputs(hidden, encoder_outputs, W_attn, b_attn, v):
    import ml_dtypes
    bf16 = ml_dtypes.bfloat16

    hidden = np.asarray(hidden, dtype=np.float32)
    enc = np.asarray(encoder_outputs, dtype=np.float32)
    W = np.asarray(W_attn, dtype=np.float32)
    b = np.asarray(b_attn, dtype=np.float32)
    v = np.asarray(v, dtype=np.float32)

    w1t = np.ascontiguousarray(W[:, :H].T).astype(bf16)
    w2t = np.ascontiguousarray(W[:, H:].T).astype(bf16)
    hidT = np.ascontiguousarray(hidden.T).astype(bf16)
    brow = b[None, :].astype(bf16)
    ones = np.ones((1, B), bf16)
    ind = np.tile(np.eye(B, dtype=np.float32), (4, 512 // B)).astype(bf16)
    vcol = np.ascontiguousarray(v.reshape(MC, P).T)

    in_maps = []
    for c in range(NCORES):
        shard = enc[c * TC:(c + 1) * TC]
        encT = np.ascontiguousarray(
            shard.reshape(TC * B, H).T).astype(bf16)
        in_maps.append({
            "enct": encT, "w2t": w2t, "w1t": w1t, "hidt": hidT,
            "brow": brow, "ones": ones, "ind": ind, "vcol": vcol,
        })
    return in_maps


def _assemble(results):
    out = np.empty((B, 1, T), np.float32)
    for c in range(NCORES):
        s = results[c]["scores"].reshape(4, 4, NT // NTG, NTG, 4, B)
        s = np.stack([s[:, j, :, :, j, :] for j in range(4)], axis=1)
        s = s.transpose(2, 3, 0, 1, 4)
        out[:, 0, c * TC:(c + 1) * TC] = s.reshape(TC, B).T
    return out


def run(in_maps, trace=False, **kw):
    from concourse.bass_utils import run_bass_kernel_spmd

    if "nc" not in _CACHE:
        _CACHE["nc"] = _build_nc()
    nc = _CACHE["nc"]
    return run_bass_kernel_spmd(
        nc, in_maps, list(range(NCORES)), trace=trace, **kw
    )


def kernel(hidden, encoder_outputs, W_attn, b_attn, v):
    in_maps = _prep_inputs(hidden, encoder_outputs, W_attn, b_attn, v)
    br = run(in_maps)
    return _assemble(br.results)


# revision 9
# speedup vs baseline: 1.2926x; 1.2926x over previous
"""Bass/Trainium2 kernel for nn_Attention_1245540515949.

Reference computation (B=32, T=4096, H=512), fp32 inputs:
    cat    = concat([broadcast(hidden), enc], -1)          # [B,T,2H]
    energy = softmax(cat @ W_attn.T + b_attn, axis=0)      # batch-dim softmax!
    scores = relu(einsum('h,bth->bt', v, energy))[:, None] # [B,1,T]

Strategy: shard T across the 8 cores (the batch softmax stays core-local).
Per core the 512*32 = 16384 (t,b) columns (b inner) are processed in 16
blocks of 1024 columns (32 t each):

  E[h,(t,b)] = W2T.T @ enc                   bf16 matmuls, k-chunked (4x128),
       + A'[b,h]                             + K=32 "indicator" matmuls
                                             (row-packed via tile_position);
                                             A' = hidden@W1.T + b_attn is
                                             computed on the HOST (exact f32)
  X   = exp(E)                               ScalarE, one [128,1024] ACT per
                                             2-bank PSUM tile
  den[t,h] = sum_b X                         DVE: ONE segmented reduce per
                                             block ([p,128,32] -> [p,128])
  u[h,t]   = v[h] / den[t,h]                 DVE reciprocal_approx_fast + mul
  scores   = u.T @ X                         M=32,N=512 matmul chains; the 4
                                             col-groups of one PSUM bank hold
                                             2 blocks' scores (diagonal slots
                                             valid). Issued with a 2-block lag
                                             so the PE never waits on the den
                                             path.
  copy + DMA out                             DVE [128,512] copy -> bf16;
                                             relu + diagonal extract on HOST

enc ships as bf16 [H, cols] pre-arranged so each DMA is a 512 KiB transfer
with 4 KiB contiguous per partition (vs 128 KiB/1 KiB in the old layout,
which capped HBM at ~160 GB/s). HBM traffic ~17 MiB/core.
"""

import numpy as np

B, T, H = 32, 4096, 512
NCORES = 8
TC = T // NCORES          # 512 t-values per core
P = 128                   # partitions
NCOL = TC * B             # 16384 (t,b) columns per core
NBLK = NCOL // 1024       # 16 blocks of 1024 columns (32 t each)
NPAIR = NBLK // 2         # 8 block-pairs (DMA + scores-PSUM granularity)

_CACHE = {}


def _build_nc():
    import concourse.mybir as mybir
    from concourse.bacc import Bacc
    from concourse.tile import TileContext

    f32 = mybir.dt.float32
    bf16 = mybir.dt.bfloat16
    AF = mybir.ActivationFunctionType
    AX = mybir.AxisListType

    nc = Bacc()

    encb = nc.declare_dram_parameter("encb", [P, NPAIR * 8192], bf16,
                                     isOutput=False)
    w2t = nc.declare_dram_parameter("w2t", [H, H], bf16, isOutput=False)
    aprep = nc.declare_dram_parameter("aprep", [P, 512], bf16, isOutput=False)
    ind = nc.declare_dram_parameter("ind", [P, 512], bf16, isOutput=False)
    vrep = nc.declare_dram_parameter("vrep", [P, P], f32, isOutput=False)
    out = nc.declare_dram_parameter("scores", [P, NPAIR * 512], bf16,
                                    isOutput=True)

    encv = encb.rearrange("p (pr kc j n) -> p pr kc j n", pr=NPAIR, kc=4, j=2)

    with TileContext(nc) as tc:
        with (
            tc.tile_pool(name="consts", bufs=1) as consts,
            tc.tile_pool(name="enc", bufs=NPAIR) as encp,
            tc.tile_pool(name="xs", bufs=4) as xp,
            tc.tile_pool(name="dens", bufs=6) as dp,
            tc.tile_pool(name="us", bufs=4) as up,
            tc.tile_pool(name="scb", bufs=3) as scb,
            tc.tile_pool(name="eps", bufs=3, space="PSUM") as eps,
            tc.tile_pool(name="scps", bufs=2, space="PSUM") as scps,
        ):
            # ---- constants into SBUF ----
            w2_sb = []
            for kc in range(4):
                t_ = consts.tile([P, H], bf16, name=f"w2t{kc}")
                nc.sync.dma_start(out=t_, in_=w2t[kc * P:(kc + 1) * P, :])
                w2_sb.append(t_)
            aprep_sb = consts.tile([P, 512], bf16, name="aprep")
            nc.sync.dma_start(out=aprep_sb, in_=aprep[:, :])
            ind_sb = consts.tile([P, 512], bf16, name="ind")
            nc.sync.dma_start(out=ind_sb, in_=ind[:, :])
            vrep_sb = consts.tile([P, P], f32, name="vrep")
            nc.sync.dma_start(out=vrep_sb, in_=vrep[:, :])
            # prewarm the exp table set so ACT_TABLE_LOAD overlaps the
            # enc prefetch instead of stalling the first tile
            warm = consts.tile([1, 1], f32, name="actwarm")
            nc.scalar.activation(out=warm, in_=vrep_sb[0:1, 0:1], func=AF.Exp)

            # ---- main loop (scores lag 2 blocks behind the E/X pipeline
            #      so the PE instruction stream never stalls on den/u) ----
            x_hist = [None] * NBLK
            u_hist = [None] * NBLK
            enc_cur = None
            sc_ps = None
            for it in range(NBLK + 2):
                if it < NBLK:
                    blk = it
                    pair, j = blk // 2, blk % 2
                    if j == 0:
                        etile = encp.tile([P, 8192], bf16, tag="enc")
                        for kc in range(4):
                            nc.sync.dma_start(
                                out=etile[:, kc * 2048:(kc + 1) * 2048],
                                in_=encv[:, pair, kc],
                            )
                        enc_cur = etile.rearrange(
                            "p (kc j n) -> p kc j n", kc=4, j=2)

                    x_all = xp.tile([P, 4096], bf16, tag="x")
                    x_hist[blk] = x_all
                    # mc-pair structure: 4 consecutive kc-MMs accumulate into
                    # ONE psum bank (avoids per-MM bank cycling, a PE
                    # micro-idle trap), and the K=32 A'-closers of two mc
                    # tiles are issued adjacently so their disjoint PE row
                    # groups overlap.
                    for mp in range(2):
                        mcs = (2 * mp, 2 * mp + 1)
                        ep_of = {}
                        for mc in mcs:
                            ep = eps.tile([P, 1024], f32, tag="e")
                            ep_of[mc] = ep
                            for half in range(2):
                                for kc in range(4):
                                    nc.tensor.matmul(
                                        out=ep[:, half * 512:
                                               (half + 1) * 512],
                                        lhsT=w2_sb[kc][:, mc * P:
                                                       (mc + 1) * P],
                                        rhs=enc_cur[:, kc, j, half * 512:
                                                    (half + 1) * 512],
                                        start=(kc == 0), stop=False,
                                    )
                        for half in range(2):
                            for mc in mcs:
                                nc.tensor.matmul(
                                    out=ep_of[mc][:, half * 512:
                                                  (half + 1) * 512],
                                    lhsT=aprep_sb[32 * mc:32 * (mc + 1),
                                                  mc * P:(mc + 1) * P],
                                    rhs=ind_sb[32 * mc:32 * (mc + 1), :],
                                    start=False, stop=True,
                                    tile_position=(32 * mc, 0),
                                )
                        for mc in mcs:
                            nc.scalar.activation(
                                out=x_all[:, mc * 1024:(mc + 1) * 1024],
                                in_=ep_of[mc], func=AF.Exp,
                            )

                    # den path: one segmented reduce, fast recip, u = v/den
                    x3 = x_all.rearrange("p (mt b) -> p mt b", b=32)
                    den = dp.tile([P, P], f32, tag="den")
                    nc.vector.reduce_sum(out=den, in_=x3, axis=AX.X)
                    rden = dp.tile([P, P], f32, tag="rden")
                    nc.vector.reciprocal_approx_fast(out=rden, in_=den)
                    u = up.tile([P, P], bf16, tag="u")
                    nc.vector.tensor_mul(out=u, in0=rden, in1=vrep_sb)
                    u_hist[blk] = u

                sk = it - 2
                if sk >= 0:
                    spair, sj = sk // 2, sk % 2
                    if sj == 0:
                        sc_ps = scps.tile([P, 512], f32, tag="sc")
                    # per half one M=32 N=512 chain over mc; col-group
                    # g = 2*sj + half of the pair's bank; valid slots are
                    # out[32*g + 16*half + jj, 32*jj + b]
                    for half in range(2):
                        g = sj * 2 + half
                        for mc in range(4):
                            nc.tensor.matmul(
                                out=sc_ps[32 * g:32 * (g + 1), :],
                                lhsT=u_hist[sk][:, mc * 32:(mc + 1) * 32],
                                rhs=x_hist[sk][:, mc * 1024 + half * 512:
                                               mc * 1024 + half * 512 + 512],
                                start=(mc == 0), stop=(mc == 3),
                                tile_position=(0, 32 * g),
                            )
                    if sj == 1:
                        ssb = scb.tile([P, 512], bf16, tag="ssb")
                        nc.vector.tensor_copy(out=ssb, in_=sc_ps)
                        nc.sync.dma_start(
                            out=out[:, spair * 512:(spair + 1) * 512],
                            in_=ssb,
                        )

    nc.compile()
    return nc


def _prep_inputs(hidden, encoder_outputs, W_attn, b_attn, v):
    """Host-side shard + layout prep. Returns in_maps for the 8 cores."""
    import ml_dtypes
    bf16 = ml_dtypes.bfloat16

    hidden = np.asarray(hidden, dtype=np.float32)
    enc = np.asarray(encoder_outputs, dtype=np.float32)
    W = np.asarray(W_attn, dtype=np.float32)
    b = np.asarray(b_attn, dtype=np.float32)
    v = np.asarray(v, dtype=np.float32)

    w2t = np.ascontiguousarray(W[:, H:].T).astype(bf16)      # [h_in, h_out]
    # A' = hidden @ W1.T + b_attn, exact on host, replicated to the 4
    # 32-row groups used by the indicator matmuls
    apr = hidden @ W[:, :H].T + b[None, :]                   # [B, H]
    aprep = np.tile(apr, (4, 1)).astype(bf16)                # [128, 512]
    ind = np.tile(np.eye(B, dtype=np.float32), (4, 512 // B)).astype(bf16)
    vcol = np.ascontiguousarray(v.reshape(4, P).T)           # [P, 4] f32
    vrep = np.repeat(vcol, 32, axis=1).astype(np.float32)    # [P, 128]

    in_maps = []
    for c in range(NCORES):
        shard = enc[c * TC:(c + 1) * TC]                     # [TC, B, H]
        encT = shard.reshape(NCOL, H).T                      # [H, NCOL]
        encb = np.ascontiguousarray(
            encT.reshape(4, P, NPAIR, 2, 1024)
                .transpose(1, 2, 0, 3, 4).reshape(P, NPAIR * 8192)
        ).astype(bf16)
        in_maps.append({
            "encb": encb, "w2t": w2t, "aprep": aprep, "ind": ind,
            "vrep": vrep,
        })
    return in_maps


def _assemble(results):
    """results: per-core dicts with 'scores' [128, NPAIR*512] bf16.

    Column layout: col = pair*512 + 32*jj + b. Valid rows per quarter q
    (t = 64*pair + 16*q + jj): q=0 -> row jj, q=1 -> 48+jj, q=2 -> 64+jj,
    q=3 -> 112+jj.
    """
    rowbase = (0, 48, 64, 112)
    out = np.empty((B, 1, T), np.float32)
    for c in range(NCORES):
        s = np.asarray(results[c]["scores"], dtype=np.float32)
        s4 = s.reshape(P, NPAIR, 16, B)                      # [row,pair,jj,b]
        for q in range(4):
            for jj in range(16):
                vals = s4[rowbase[q] + jj, :, jj, :]         # [pair, b]
                t0 = c * TC + 16 * q + jj
                out[:, 0, t0:t0 + 64 * NPAIR:64] = np.maximum(vals, 0.0).T
    return out


def run(in_maps, trace=False, **kw):
    from concourse.bass_utils import run_bass_kernel_spmd

    if "nc" not in _CACHE:
        _CACHE["nc"] = _build_nc()
    nc = _CACHE["nc"]
    return run_bass_kernel_spmd(
        nc, in_maps, list(range(NCORES)), trace=trace, **kw
    )


def kernel(hidden, encoder_outputs, W_attn, b_attn, v):
    in_maps = _prep_inputs(hidden, encoder_outputs, W_attn, b_attn, v)
    br = run(in_maps)
    return _assemble(br.results)
